# revision 1
# baseline (speedup 1.0000x reference)
"""Distributed Trainium2 Bass kernel for nn_ActorGCN (GNN message passing), v2.

8 NeuronCores, SPMD, node-octile sharding. Structure as v1 (edge MLP over a
src-sorted bucket-padded stream; GCN aggregations via linearity A@(hW) ==
(A@h)@W with GPSIMD ap_gather against octile-split tables), with:
  - zero non-contiguous HBM DMAs: the edge-MLP second matmul is split into
    the two pair halves so every pair/quad interleave is partition-preserving
    (DVE strided SBUF writes), single contiguous DMA per HBM buffer;
  - MP2 as ONE d=4 gather pass (quad tables) instead of two d=2 passes;
  - h2 streamed through SBUF chunks (no DRAM round-trip);
  - inputs packed into three dtype blobs (3 input tensors instead of 25);
  - per-node scalars (1/cnt, dinv, pad-correction, x) host-folded into two
    fused DVE ops; final 64-float cross-core reduce + Wo/bo on host.
"""
import sys
import numpy as np

sys.path.insert(0, "/opt/trn_rl_repo")

N = 50000
E = 1600000
NCORE = 8
NSH = N // NCORE            # 6250
NSHP = NSH + 6              # 6256 (mult of 16)
F0 = 32
H = 128
F2 = 64

B0 = [4, 8, 12, 16, 20, 24, 28, 32, 36, 40, 44, 48, 56, 64, 80, 96, 128]
B1 = [1, 2, 3, 4, 5, 6, 7, 8, 10, 12, 16, 20, 24, 32, 48, 64]

SC0 = 6144
SC1 = 2048

PERM32 = np.concatenate([np.arange(0, F0, 2), np.arange(1, F0, 2)])
PERM64 = np.concatenate([np.arange(j, F2, 4) for j in range(4)])


def _bucket_vec(deg, B):
    K = np.full(deg.shape, B[-1], np.int64)
    for b in reversed(B):
        K[deg <= b] = b
    assert (deg <= K).all()
    return K


def _round8(v):
    return (int(v) + 7) // 8 * 8


def _wrap16(vals, parts, base_part=0):
    n = len(vals)
    assert n % 16 == 0
    w = n // 16
    out = np.zeros((parts, w), np.int16)
    out[base_part:base_part + 16, :] = np.asarray(vals, np.int16).reshape(w, 16).T
    return out


def _chunks(layout, cap, align=8):
    # keep every chunk's slot span (b*m) a multiple of 16 so int16 index
    # columns (wrapped 16-per-column) slice cleanly
    out = []
    node_off = 0
    slot_off = 0
    for b, cnt in layout:
        left = cnt
        mstep = max(align, (cap // b) // align * align)
        while left > 0:
            m = min(left, mstep)
            assert (b * m) % 16 == 0 and slot_off % 16 == 0
            out.append((b, m, node_off, slot_off))
            node_off += m
            slot_off += b * m
            left -= m
    return out


def _pack(arrs, dtype):
    """arrs: list of (name, 2-D array). Returns (offmap, flat[1, L])."""
    offmap, flat, off = {}, [], 0
    for name, a in arrs:
        a = np.ascontiguousarray(np.asarray(a).astype(dtype, copy=False))
        P, n = a.shape
        offmap[name] = (off, P, n)
        flat.append(a.reshape(-1))
        off += P * n
    return offmap, np.concatenate(flat).reshape(1, -1)


def host_prep(inputs):
    import ml_dtypes
    bft = ml_dtypes.bfloat16
    x = np.asarray(inputs["x"], np.float32)
    ei = np.asarray(inputs["edge_index"])
    ea = np.asarray(inputs["edge_attr"], np.float32)
    src = ei[0].astype(np.int64)
    dst = ei[1].astype(np.int64)
    W = {k: np.asarray(inputs[k], np.float32) for k in
         ["W_e1", "b_e1", "W_e2", "b_e2", "W_g1", "b_g1", "W_g2", "b_g2",
          "W_a", "b_a", "W_o", "b_o"]}

    We2p = np.zeros((H, F0), np.float32); We2p[:, :30] = W["W_e2"]
    We2p = We2p[:, PERM32]
    be2p = np.zeros((F0,), np.float32); be2p[:30] = W["b_e2"]
    be2p = be2p[PERM32]
    Wg1p = np.zeros((F0, H), np.float32); Wg1p[:30] = W["W_g1"]
    Wg1p = Wg1p[PERM32]
    Wg2p = W["W_g2"][:, PERM64]
    bg2p = W["b_g2"][PERM64]
    Wap = W["W_a"][PERM64]
    bg2_j = np.stack([bg2p[16 * j:16 * (j + 1)] for j in range(4)], 1)  # [16,4]
    # vpad = edge-MLP output of an all-zero (padding) edge slot
    vpad = np.maximum(We2p.T @ np.maximum(W["b_e1"], 0.0) + be2p, 0.0)  # [32]

    deg_dst = np.bincount(dst, minlength=N).astype(np.float32)
    dinv_all = 1.0 / np.sqrt(deg_dst + 1.0)

    # ---------- MP0 grids ----------
    per_core0 = []
    for c in range(NCORE):
        lo = c * NSH
        eidx = np.nonzero((src >= lo) & (src < lo + NSH))[0]
        s_loc = src[eidx] - lo
        order = np.argsort(s_loc, kind="stable")
        eidx = eidx[order]
        deg = np.bincount(s_loc, minlength=NSH)
        K = _bucket_vec(np.maximum(deg, 1), B0)
        per_core0.append((eidx, deg, K))
    bc0 = {b: _round8(max(int((K == b).sum()) for _, _, K in per_core0))
           for b in B0}
    layout0 = [(b, bc0[b]) for b in B0 if bc0[b] > 0]
    NG0 = sum(cnt for _, cnt in layout0)
    slots0 = sum(b * cnt for b, cnt in layout0)
    plan0 = _chunks(layout0, SC0)
    NG0P = _round8(NG0)

    # ---------- MP1/MP2 grids ----------
    per_cg = {}
    for c in range(NCORE):
        lo = c * NSH
        eidx = np.nonzero((dst >= lo) & (dst < lo + NSH))[0]
        g_of = src[eidx] // NSH
        for g in range(NCORE):
            e2 = eidx[g_of == g]
            d_loc = dst[e2] - lo
            order = np.argsort(d_loc, kind="stable")
            e2 = e2[order]
            sdeg = np.bincount(d_loc, minlength=NSH)
            K = _bucket_vec(np.maximum(sdeg, 1), B1)
            per_cg[(c, g)] = (e2, sdeg, K)
    def _round16(v):
        return (int(v) + 15) // 16 * 16
    bc1 = {b: _round16(max(int((K == b).sum()) for _, _, K in per_cg.values()))
           for b in B1}
    layout1 = [(b, bc1[b]) for b in B1 if bc1[b] > 0]
    NG1 = sum(cnt for _, cnt in layout1)
    NG1T = _round8(NG1)
    slots1 = sum(b * cnt for b, cnt in layout1)
    plan1 = _chunks(layout1, SC1, align=16)

    sel = np.zeros((128, 16), np.float32)
    for g in range(NCORE):
        sel[16 * g + np.arange(16), np.arange(16)] = 1.0
    Waq = np.concatenate([Wap[16 * j:16 * (j + 1)] for j in range(4)], 1)

    in_maps = []
    offmaps = None
    for c in range(NCORE):
        lo = c * NSH
        # ---- MP0 stream ----
        eidx, deg, K0 = per_core0[c]
        grid_nodes = np.full(NG0, -1, np.int64)
        pos = 0
        for b, cnt in layout0:
            nodes = np.nonzero(K0 == b)[0]
            grid_nodes[pos:pos + len(nodes)] = nodes
            pos += cnt
        run_b = np.concatenate([np.full(cnt, b) for b, cnt in layout0])
        run_start = np.concatenate([[0], np.cumsum(run_b)[:-1]])
        gp_of_node = np.zeros(NSH, np.int64)
        valid = grid_nodes >= 0
        gp_of_node[grid_nodes[valid]] = np.nonzero(valid)[0]
        s_loc = src[eidx] - lo
        first = np.concatenate([[0], np.cumsum(deg)[:-1]])
        within = np.arange(len(eidx)) - first[s_loc]
        slot = run_start[gp_of_node[s_loc]] + within
        ea_pad = np.zeros((slots0, 16), np.float32)
        ea_pad[slot] = ea[eidx]
        ea16 = np.ascontiguousarray(ea_pad.T)              # [16, slots0]
        permv = np.zeros(NSHP, np.int64)
        permv[:NSH] = gp_of_node
        perm0 = _wrap16(permv, 16)                          # [16, NSHP//16]

        # ---- host-folded per-node scalars ----
        cnt_n = deg.astype(np.float32)
        k_n = K0.astype(np.float32)
        alpha = 1.0 / np.maximum(cnt_n, 1.0)
        dinv = dinv_all[lo:lo + NSH]
        xp = np.zeros((F0, NSH), np.float32)
        xp[:30] = x[lo:lo + NSH].T
        xp = xp[PERM32]                                     # [32, NSH]
        A16 = (alpha * dinv)[None, :].repeat(16, 0)         # [16, NSH]
        Bx32 = ((k_n - cnt_n) * alpha)[None, :] * vpad[:, None]
        Bx32 = (Bx32 - xp) * dinv[None, :]                  # [32, NSH]
        # pair-interleave: Bxp[p, n, j] = Bx32[p + 16j, n]
        Bxp = np.stack([Bx32[0:16], Bx32[16:32]], axis=2)   # [16, NSH, 2]
        Bxp = np.ascontiguousarray(Bxp).reshape(16, NSH * 2)
        dinv16 = dinv[None, :].repeat(16, 0)                # [16, NSH]

        # ---- MP1/2 idx + perm ----
        idx1 = np.zeros((128, slots1 // 16), np.int16)
        perm1 = np.zeros((128, NSHP // 16), np.int16)
        for g in range(NCORE):
            e2, sdeg, K1 = per_cg[(c, g)]
            gn = np.full(NG1, -1, np.int64)
            pos = 0
            for b, cnt in layout1:
                nodes = np.nonzero(K1 == b)[0]
                gn[pos:pos + len(nodes)] = nodes
                pos += cnt
            run_b1 = np.concatenate([np.full(cnt, b) for b, cnt in layout1])
            rs1 = np.concatenate([[0], np.cumsum(run_b1)[:-1]])
            gpn = np.zeros(NSH, np.int64)
            vv = gn >= 0
            gpn[gn[vv]] = np.nonzero(vv)[0]
            d_loc = dst[e2] - lo
            first = np.concatenate([[0], np.cumsum(sdeg)[:-1]])
            within = np.arange(len(e2)) - first[d_loc]
            slotv = np.full(slots1, NSH, np.int64)
            slotv[rs1[gpn[d_loc]] + within] = src[e2] - g * NSH
            idx1 += _wrap16(slotv, 128, base_part=16 * g)
            pv = np.zeros(NSHP, np.int64)
            pv[:NSH] = gpn
            perm1 += _wrap16(pv, 128, base_part=16 * g)

        o16, b16f = _pack([
            ("ea", ea16), ("We1", W["W_e1"]),
            ("We2a", We2p[:, 0:16]), ("We2b", We2p[:, 16:32]),
            ("Wg1a", Wg1p[0:16]), ("Wg1b", Wg1p[16:32]), ("Wg2", Wg2p),
            ("Waq", Waq), ("sel", sel), ("dinv16", dinv16),
        ], bft)
        o32, b32f = _pack([
            ("A16", A16), ("Bxp", Bxp),
            ("be1", W["b_e1"].reshape(H, 1)),
            ("be2a", be2p[0:16].reshape(16, 1)),
            ("be2b", be2p[16:32].reshape(16, 1)),
            ("bg1", W["b_g1"].reshape(H, 1)),
            ("bg2j", bg2_j), ("ba", W["b_a"].reshape(F2, 1)),
        ], np.float32)
        oi, bif = _pack([
            ("perm0", perm0), ("idx1", idx1), ("perm1", perm1),
        ], np.int16)
        in_maps.append({"b16": b16f, "b32": b32f, "bi": bif})
        if offmaps is None:
            offmaps = (o16, o32, oi, b16f.shape[1], b32f.shape[1], bif.shape[1])

    plan = dict(layout0=layout0, NG0=NG0, NG0P=NG0P, slots0=slots0,
                plan0=plan0, layout1=layout1, NG1=NG1, NG1T=NG1T,
                slots1=slots1, plan1=plan1, offmaps=offmaps)
    fin = (W["W_o"], W["b_o"])
    return in_maps, plan, fin


def build(plan):
    from concourse import bacc, tile
    from concourse.bass import mybir
    dt = mybir.dt
    AF = mybir.ActivationFunctionType
    ALU = mybir.AluOpType
    X = mybir.AxisListType.X

    NG0P, slots0, plan0 = plan["NG0P"], plan["slots0"], plan["plan0"]
    NG1T, slots1, plan1 = plan["NG1T"], plan["slots1"], plan["plan1"]
    o16, o32, oi, L16, L32, LI = plan["offmaps"]

    nc = bacc.Bacc("TRN2", target_bir_lowering=False, debug=False,
                   num_devices=NCORE)

    b16 = nc.declare_dram_parameter("b16", [1, L16], dt.bfloat16, False)
    b32 = nc.declare_dram_parameter("b32", [1, L32], dt.float32, False)
    bi = nc.declare_dram_parameter("bi", [1, LI], dt.int16, False)
    out = nc.declare_dram_parameter("out", [1, F2], dt.float32, True)

    pin0 = nc.dram_tensor("pin0", [16, NSHP * 2], dt.bfloat16)
    pout0 = nc.dram_tensor("pout0", [NCORE, 16, NSHP * 2], dt.bfloat16,
                           addr_space="Shared")
    pin2 = nc.dram_tensor("pin2", [16, NSHP * 4], dt.bfloat16)
    pout2 = nc.dram_tensor("pout2", [NCORE, 16, NSHP * 4], dt.bfloat16,
                           addr_space="Shared")
    RG = [list(range(NCORE))]

    def bsl(blob, offmap, key):
        off, P, n = offmap[key]
        return blob[0:1, off:off + P * n].rearrange("a (p n) -> (a p) n", p=P)

    with tile.TileContext(nc) as tc:
        with tc.tile_pool(name="const", bufs=1) as cpool:
            def load(blob, offmap, key, dtype, pool=None, tag=None):
                off, P, n = offmap[key]
                t = (pool or cpool).tile([P, n], dtype, tag=tag or key)
                nc.sync.dma_start(out=t[:], in_=bsl(blob, offmap, key))
                return t

            We1_s = load(b16, o16, "We1", dt.bfloat16)
            We2a_s = load(b16, o16, "We2a", dt.bfloat16)
            We2b_s = load(b16, o16, "We2b", dt.bfloat16)
            sel_s = load(b16, o16, "sel", dt.bfloat16)
            dinv16_s = load(b16, o16, "dinv16", dt.bfloat16)
            be1_s = load(b32, o32, "be1", dt.float32)
            be2a_s = load(b32, o32, "be2a", dt.float32)
            be2b_s = load(b32, o32, "be2b", dt.float32)
            idx1_s = load(bi, oi, "idx1", dt.int16)
            perm1_s = load(bi, oi, "perm1", dt.int16)

            ea_off = o16["ea"][0]
            ea_ap = b16[0:1, ea_off:ea_off + 16 * slots0].rearrange(
                "a (p s) -> (a p) s", p=16)
            A_off = o32["A16"][0]
            A_ap = b32[0:1, A_off:A_off + 16 * NSH].rearrange(
                "a (p n) -> (a p) n", p=16)
            Bx_off = o32["Bxp"][0]
            Bx_ap = b32[0:1, Bx_off:Bx_off + 16 * NSH * 2].rearrange(
                "a (p n) -> (a p) n", p=16)

            # ---------------- Phase 0: edge MLP + segment reduce ----------
            with tc.tile_pool(name="pg", bufs=1) as pg:
                sum0a = pg.tile([16, NG0P], dt.bfloat16, tag="sum0a")
                sum0b = pg.tile([16, NG0P], dt.bfloat16, tag="sum0b")
                with (
                    tc.tile_pool(name="mlp", bufs=2) as mp,
                    tc.tile_pool(name="ps0", bufs=2, space="PSUM") as ps0,
                ):
                    for (b, m, node_off, slot_off) in plan0:
                        n = b * m
                        eat = mp.tile([16, SC0], dt.bfloat16, tag="ea")
                        nc.sync.dma_start(
                            out=eat[:, :n],
                            in_=ea_ap[:, slot_off:slot_off + n])
                        ef1 = mp.tile([H, SC0], dt.bfloat16, tag="ef1")
                        for j in range(0, n, 512):
                            w = min(512, n - j)
                            pt = ps0.tile([H, 512], dt.float32, tag="ps1")
                            nc.tensor.matmul(out=pt[:, :w], lhsT=We1_s[:],
                                             rhs=eat[:, j:j + w],
                                             start=True, stop=True)
                            nc.scalar.activation(out=ef1[:, j:j + w],
                                                 in_=pt[:, :w],
                                                 func=AF.Relu,
                                                 bias=be1_s[:])
                        ef2a = mp.tile([16, SC0], dt.bfloat16, tag="ef2a")
                        ef2b = mp.tile([16, SC0], dt.bfloat16, tag="ef2b")
                        # batch same-weight matmuls: one lhsT swap per half
                        for j in range(0, n, 512):
                            w = min(512, n - j)
                            pta = ps0.tile([16, 512], dt.float32, tag="ps2a")
                            nc.tensor.matmul(out=pta[:, :w], lhsT=We2a_s[:],
                                             rhs=ef1[:, j:j + w],
                                             start=True, stop=True)
                            nc.scalar.activation(out=ef2a[:, j:j + w],
                                                 in_=pta[:, :w],
                                                 func=AF.Relu,
                                                 bias=be2a_s[:])
                        for j in range(0, n, 512):
                            w = min(512, n - j)
                            ptb = ps0.tile([16, 512], dt.float32, tag="ps2b")
                            nc.tensor.matmul(out=ptb[:, :w], lhsT=We2b_s[:],
                                             rhs=ef1[:, j:j + w],
                                             start=True, stop=True)
                            nc.vector.tensor_scalar(
                                out=ef2b[:, j:j + w], in0=ptb[:, :w],
                                scalar1=be2b_s[:], scalar2=0.0,
                                op0=ALU.add, op1=ALU.max)
                        with nc.allow_low_precision("bf16 run sums"):
                            nc.vector.tensor_reduce(
                                out=sum0a[:, node_off:node_off + m],
                                in_=ef2a[:, :n].rearrange(
                                    "p (m b) -> p m b", m=m),
                                axis=X, op=ALU.add)
                            nc.vector.tensor_reduce(
                                out=sum0b[:, node_off:node_off + m],
                                in_=ef2b[:, :n].rearrange(
                                    "p (m b) -> p m b", m=m),
                                axis=X, op=ALU.add)
                # interleave halves into pairs (partition-preserving)
                pairg = pg.tile([16, NG0P, 2], dt.bfloat16, tag="pairg")
                nc.vector.tensor_copy(out=pairg[:, :, 0], in_=sum0a[:])
                nc.vector.tensor_copy(out=pairg[:, :, 1], in_=sum0b[:])
                perm0_s = load(bi, oi, "perm0", dt.int16, pg)
                # chunked canonical permute + host-folded tail, streamed out
                with tc.tile_pool(name="tl", bufs=2) as tl:
                    for o in range(0, NSHP, 1024):
                        wg = min(1024, NSHP - o)
                        w = max(0, min(1024, NSH - o))
                        canc = tl.tile([16, 1024, 2], dt.bfloat16, tag="canc")
                        with nc.allow_low_precision("bf16 permute"):
                            nc.gpsimd.ap_gather(
                                out_ap=canc[:, :wg, :], in_ap=pairg[:],
                                idxs_ap=perm0_s[:, o // 16:(o + wg) // 16],
                                channels=16, num_elems=NG0P, d=2,
                                num_idxs=wg)
                        pb = tl.tile([16, 1024, 2], dt.bfloat16, tag="pb")
                        if w > 0:
                            A_c = tl.tile([16, 1024], dt.float32, tag="A")
                            nc.sync.dma_start(out=A_c[:, :w],
                                              in_=A_ap[:, o:o + w])
                            Bx_c = tl.tile([16, 1024, 2], dt.float32,
                                           tag="Bx")
                            nc.sync.dma_start(
                                out=Bx_c[:, :w, :].rearrange(
                                    "p n d -> p (n d)"),
                                in_=Bx_ap[:, 2 * o:2 * (o + w)])
                            t_c = tl.tile([16, 1024, 2], dt.float32, tag="t")
                            nc.vector.tensor_tensor(
                                out=t_c[:, :w, :], in0=canc[:, :w, :],
                                in1=A_c[:, :w].unsqueeze(-1)
                                .broadcast_to([16, w, 2]), op=ALU.mult)
                            nc.vector.tensor_tensor(
                                out=pb[:, :w, :], in0=t_c[:, :w, :],
                                in1=Bx_c[:, :w, :], op=ALU.subtract)
                        if wg > w:
                            zz = pb[:, w:wg, :].rearrange("p n d -> p (n d)")
                            nc.vector.memset(zz, 0.0)
                        nc.sync.dma_start(
                            out=pin0[:, 2 * o:2 * (o + wg)],
                            in_=pb[:, :wg, :].rearrange("p n d -> p (n d)"))
            nc.gpsimd.collective_compute(
                "AllGather", ALU.bypass, replica_groups=RG,
                ins=[pin0[:].opt()], outs=[pout0[:].opt()])

            # ---------------- MP1 ----------------
            with tc.tile_pool(name="mpA", bufs=1) as mpA:
                qgrid = mpA.tile([128, NG1T, 2], dt.bfloat16, tag="qgrid")
                with (
                    tc.tile_pool(name="gt", bufs=1) as gt,
                    tc.tile_pool(name="gch", bufs=2) as gchp,
                ):
                    table = gt.tile([128, NSHP, 2], dt.bfloat16, tag="table")
                    for g in range(NCORE):
                        nc.sync.dma_start(
                            out=table[16 * g:16 * (g + 1), :, :],
                            in_=pout0[g, :, :].rearrange("p (n d) -> p n d",
                                                         d=2))
                    with nc.allow_low_precision("bf16 grid"):
                        for (b, m, node_off, slot_off) in plan1:
                            n = b * m
                            gch = gchp.tile([128, SC1, 2], dt.bfloat16,
                                            tag="gch")
                            nc.gpsimd.ap_gather(
                                out_ap=gch[:, :n, :], in_ap=table[:],
                                idxs_ap=idx1_s[:, slot_off // 16:
                                               (slot_off + n) // 16],
                                channels=128, num_elems=NSHP, d=2,
                                num_idxs=n)
                            nc.vector.tensor_reduce(
                                out=qgrid[:, node_off:node_off + m, :],
                                in_=gch[:, :n, :].rearrange(
                                    "p (m b) d -> p m d b", m=m),
                                axis=X, op=ALU.add)
                with (
                    tc.tile_pool(name="mpB", bufs=1) as mpB,
                    tc.tile_pool(name="ps1p", bufs=2, space="PSUM") as ps1p,
                    tc.tile_pool(name="sm", bufs=2) as sm,
                ):
                    ppair = mpB.tile([16, NSHP, 2], dt.bfloat16, tag="ppair")
                    nc.sync.dma_start(
                        out=ppair[:].rearrange("p n d -> p (n d)"),
                        in_=pin0[:])
                    m1 = mpB.tile([16, NSH, 2], dt.bfloat16, tag="m1")
                    for o in range(0, NSH, 512):
                        w = min(512, NSH - o)
                        wg = min(512, NSHP - o)
                        qc = sm.tile([128, 512, 2], dt.bfloat16, tag="qc")
                        with nc.allow_low_precision("bf16 permute"):
                            nc.gpsimd.ap_gather(
                                out_ap=qc[:, :wg, :], in_ap=qgrid[:],
                                idxs_ap=perm1_s[:, o // 16:(o + wg) // 16],
                                channels=128, num_elems=NG1T, d=2,
                                num_idxs=wg)
                        qp = ps1p.tile([16, 1024], dt.float32, tag="gs")
                        for k in range(0, w, 256):
                            kw = min(256, w - k)
                            nc.tensor.matmul(
                                out=qp[:, 2 * k:2 * (k + kw)], lhsT=sel_s[:],
                                rhs=qc[:, k:k + kw, :].rearrange(
                                    "p n d -> p (n d)"),
                                start=True, stop=True)
                        t = sm.tile([16, 1024], dt.float32, tag="t")
                        nc.vector.tensor_tensor(
                            out=t[:, :2 * w], in0=qp[:, :2 * w],
                            in1=ppair[:, o:o + w, :].rearrange(
                                "p n d -> p (n d)"),
                            op=ALU.add)
                        nc.vector.tensor_tensor(
                            out=m1[:, o:o + w, :],
                            in0=t[:, :2 * w].rearrange("p (n d) -> p n d",
                                                       d=2),
                            in1=dinv16_s[:, o:o + w].unsqueeze(-1)
                            .broadcast_to([16, w, 2]),
                            op=ALU.mult)
                    Wg1a_s = load(b16, o16, "Wg1a", dt.bfloat16, mpB)
                    Wg1b_s = load(b16, o16, "Wg1b", dt.bfloat16, mpB)
                    bg1_s = load(b32, o32, "bg1", dt.float32, mpB)
                    Wg2_s = load(b16, o16, "Wg2", dt.bfloat16, mpB)
                    h1 = mpB.tile([H, NSH], dt.bfloat16, tag="h1")
                    for o in range(0, NSH, 512):
                        w = min(512, NSH - o)
                        hp = ps1p.tile([H, 512], dt.float32, tag="h1p")
                        nc.tensor.matmul(out=hp[:, :w], lhsT=Wg1a_s[:],
                                         rhs=m1[:, o:o + w, 0],
                                         start=True, stop=False)
                        nc.tensor.matmul(out=hp[:, :w], lhsT=Wg1b_s[:],
                                         rhs=m1[:, o:o + w, 1],
                                         start=False, stop=True)
                        nc.scalar.activation(out=h1[:, o:o + w],
                                             in_=hp[:, :w],
                                             func=AF.Relu, bias=bg1_s[:])
                    # p2 = dinv * (h1.T @ Wg2p) quads, streamed to pin2
                    for o in range(0, NSH, 512):
                        w = min(512, NSH - o)
                        p2c = sm.tile([16, 512, 4], dt.bfloat16, tag="p2c")
                        for j in range(4):
                            zp = ps1p.tile([16, 512], dt.float32, tag="zp")
                            nc.tensor.matmul(
                                out=zp[:, :w],
                                lhsT=Wg2_s[:, 16 * j:16 * (j + 1)],
                                rhs=h1[:, o:o + w],
                                start=True, stop=True)
                            nc.vector.tensor_tensor(
                                out=p2c[:, :w, j], in0=zp[:, :w],
                                in1=dinv16_s[:, o:o + w], op=ALU.mult)
                        nc.sync.dma_start(
                            out=pin2[:, 4 * o:4 * (o + w)],
                            in_=p2c[:, :w, :].rearrange("p n d -> p (n d)"))
                    zt2 = mpB.tile([16, 4 * (NSHP - NSH)], dt.bfloat16,
                                   tag="zt2")
                    nc.vector.memset(zt2[:], 0.0)
                    nc.sync.dma_start(out=pin2[:, 4 * NSH:], in_=zt2[:])
            nc.gpsimd.collective_compute(
                "AllGather", ALU.bypass, replica_groups=RG,
                ins=[pin2[:].opt()], outs=[pout2[:].opt()])

            # ---------------- MP2 (single d=4 pass) ----------------
            with tc.tile_pool(name="mpA2", bufs=1) as mpA2:
                qgrid2 = mpA2.tile([128, NG1T, 4], dt.bfloat16, tag="qgrid2")
                with (
                    tc.tile_pool(name="gt2", bufs=1) as gt2,
                    tc.tile_pool(name="gch2", bufs=2) as gchp2,
                ):
                    table2 = gt2.tile([128, NSHP, 4], dt.bfloat16,
                                      tag="table2")
                    for g in range(NCORE):
                        nc.sync.dma_start(
                            out=table2[16 * g:16 * (g + 1), :, :],
                            in_=pout2[g, :, :].rearrange("p (n d) -> p n d",
                                                         d=4))
                    with nc.allow_low_precision("bf16 grid"):
                        for (b, m, node_off, slot_off) in plan1:
                            n = b * m
                            gch = gchp2.tile([128, SC1, 4], dt.bfloat16,
                                             tag="gch2")
                            nc.gpsimd.ap_gather(
                                out_ap=gch[:, :n, :], in_ap=table2[:],
                                idxs_ap=idx1_s[:, slot_off // 16:
                                               (slot_off + n) // 16],
                                channels=128, num_elems=NSHP, d=4,
                                num_idxs=n)
                            nc.vector.tensor_reduce(
                                out=qgrid2[:, node_off:node_off + m, :],
                                in_=gch[:, :n, :].rearrange(
                                    "p (m b) d -> p m d b", m=m),
                                axis=X, op=ALU.add)
                with (
                    tc.tile_pool(name="mpB2", bufs=1) as mpB2,
                    tc.tile_pool(name="psg", bufs=1, space="PSUM") as psg,
                    tc.tile_pool(name="psa", bufs=2, space="PSUM") as psa,
                    tc.tile_pool(name="sm2", bufs=2) as sm2,
                ):
                    bg2j_s = load(b32, o32, "bg2j", dt.float32, mpB2)
                    ba_s = load(b32, o32, "ba", dt.float32, mpB2)
                    Waq_s = load(b16, o16, "Waq", dt.bfloat16, mpB2)
                    asum = mpB2.tile([F2, 1], dt.float32, tag="asum")
                    ab = mpB2.tile([F2, NSH], dt.bfloat16, tag="ab")
                    for o in range(0, NSH, 512):
                        w = min(512, NSH - o)
                        wg = min(512, NSHP - o)
                        qc2 = sm2.tile([128, 512, 4], dt.bfloat16, tag="qc2")
                        with nc.allow_low_precision("bf16 permute"):
                            nc.gpsimd.ap_gather(
                                out_ap=qc2[:, :wg, :], in_ap=qgrid2[:],
                                idxs_ap=perm1_s[:, o // 16:(o + wg) // 16],
                                channels=128, num_elems=NG1T, d=4,
                                num_idxs=wg)
                        qp = psg.tile([16, 2048], dt.float32, tag="gs2")
                        for k in range(0, w, 128):
                            kw = min(128, w - k)
                            nc.tensor.matmul(
                                out=qp[:, 4 * k:4 * (k + kw)], lhsT=sel_s[:],
                                rhs=qc2[:, k:k + kw, :].rearrange(
                                    "p n d -> p (n d)"),
                                start=True, stop=True)
                        p2l = sm2.tile([16, 512, 4], dt.bfloat16, tag="p2l")
                        nc.sync.dma_start(
                            out=p2l[:, :w, :].rearrange("p n d -> p (n d)"),
                            in_=pin2[:, 4 * o:4 * (o + w)])
                        t = sm2.tile([16, 2048], dt.float32, tag="t2")
                        nc.vector.tensor_tensor(
                            out=t[:, :4 * w], in0=qp[:, :4 * w],
                            in1=p2l[:, :w, :].rearrange("p n d -> p (n d)"),
                            op=ALU.add)
                        t3 = t[:, :4 * w].rearrange("p (n d) -> p n d", d=4)
                        nc.vector.tensor_tensor(
                            out=t3, in0=t3,
                            in1=dinv16_s[:, o:o + w].unsqueeze(-1)
                            .broadcast_to([16, w, 4]), op=ALU.mult)
                        nc.vector.tensor_tensor(
                            out=t3, in0=t3,
                            in1=bg2j_s[:].unsqueeze(1)
                            .broadcast_to([16, w, 4]),
                            op=ALU.add)
                        h2c = sm2.tile([16, 512, 4], dt.bfloat16, tag="h2c")
                        nc.vector.tensor_scalar_max(
                            out=h2c[:, :w, :].rearrange("p n d -> p (n d)"),
                            in0=t[:, :4 * w], scalar1=0.0)
                        ap_ = psa.tile([F2, 512], dt.float32, tag="ap")
                        for j in range(4):
                            nc.tensor.matmul(
                                out=ap_[:, :w],
                                lhsT=Waq_s[:, 64 * j:64 * (j + 1)],
                                rhs=h2c[:, :w, j],
                                start=(j == 0), stop=(j == 3))
                        nc.scalar.activation(out=ab[:, o:o + w],
                                             in_=ap_[:, :w],
                                             func=AF.Relu, bias=ba_s[:])
                    nc.vector.tensor_reduce(out=asum[:], in_=ab[:], axis=X,
                                            op=ALU.add)
                    nc.sync.dma_start(out=out[:].rearrange("o p -> p o"),
                                      in_=asum[:])
    nc.compile()
    return nc


def kernel(trace=False, **inputs):
    from concourse.bass_utils import run_bass_kernel_spmd
    in_maps, plan, fin = host_prep(inputs)
    nc = build(plan)
    res = run_bass_kernel_spmd(nc, in_maps, core_ids=list(range(NCORE)),
                               trace=trace)
    Wo, bo = fin
    tot = np.zeros(F2, np.float64)
    for c in range(NCORE):
        tot += np.asarray(res.results[c]["out"], np.float32).reshape(F2)
    y = (tot / N) @ Wo.astype(np.float64) + bo.astype(np.float64)
    o = y.astype(np.float32)
    if trace:
        return o, res
    return o

